# revision 11
# baseline (speedup 1.0000x reference)
"""Trainium2 Bass kernel for nn_Entropy (KDE local-entropy via histogram binning).

Contract: kernel(**inputs) takes the FULL input x (2,2,1,80,80) fp32 and
returns the FULL output (2,2,80,80) fp32, sharding internally across 8
NeuronCores (core = batch*2 + row-half of the 74x74 patch grid).

Algorithm (per core, one 47x80 input strip -> 37x74 entropy block):
  1. unsharp preprocessing (5x5 blur via PE banded matmul + free-axis tree
     adds, exact-tie-aware rounding, IEEE-reciprocal division) -> integer
     "division" image in [0,255].
  2. KDE entropy: per 7x7 patch the pairwise-exp sum collapses onto a
     256-bin histogram h; G = K @ h with the constant 256x256 kernel matrix
     K[b,b'] = exp(-(b-b')^2/12.5); ent = -(1/49) * h . log(G/(49*norm)+eps).
     h is built from a fused fp16 one-hot image (bins on partitions, both
     128-bin halves side by side) box-summed with shifted-add trees
     (7 = 4+2+1) vertically then horizontally; all counts are fp16-exact.

Pipelining layout (v2): input DMAs ride separate engine queues; the one-hot
is_equal, the vertical/horizontal shifted-add trees and stage C all run in
row-blocks so PE (G matmuls), ACT (Ln/copies) and DVE (adds/muls) overlap;
the -1/49 entropy scale is folded into the PSUM->SBUF output copy on ACT.
"""
import os
import sys

import numpy as np

for _p in ("/opt/trn_rl_repo", "/root/.axon_site/_ro/trn_rl_repo"):
    if os.path.isdir(_p) and _p not in sys.path:
        sys.path.insert(0, _p)

import concourse.bass as bass
import concourse.bacc as bacc
import concourse.tile as tile
from concourse import mybir
from concourse.bass_utils import run_bass_kernel_spmd

dt = mybir.dt
Alu = mybir.AluOpType
Act = mybir.ActivationFunctionType
f32 = np.float32

R = 7
BW = 2.5
L = R * R  # 49
NORM = f32((2.0 * np.pi * BW * BW) ** 0.5)  # C=1 -> exponent 1/2
LN_SCALE = float(f32(1.0 / (L * NORM)))
INV25 = float(f32(1.0) / f32(25.0))
NEG_INV_L = float(-(f32(1.0) / f32(L)))
MAGIC = 8388608.0  # RNE(v) == (v + 2^23) - 2^23 for 0 <= v < 2^23

# geometry
HP = 74          # patch grid cols (80 - 7 + 1)
ROWS = 43        # division-image rows needed per core (37 patch rows + 6)
PR = 37          # patch rows per core
NPIX = ROWS * 80         # 3440
NP_ = PR * HP            # 2738

# schedule knobs
BCCH = 430                           # broadcast chunk (<=512 psum cols)
OH_GROUPS = [0, 1000, 2000, NPIX]    # is_equal spans (pixel index)
V_BLOCKS = [(0, 18), (18, PR)]       # vertical-tree row blocks
H_GROUPS = [(0, 12), (12, 24), (24, PR)]   # h-tree / stage-C row groups
C_ROWS = [[(0, 6), (6, 12)], [(12, 18), (18, 24)], [(24, 30), (30, 36), (36, PR)]]

_COMPILED = None


def _host_constants():
    f16 = np.float16
    bins = np.arange(256, dtype=np.float64)
    km = np.exp(-((bins[:, None] - bins[None, :]) ** 2) / (2.0 * BW * BW))
    # kmI layout: [128, 5*128]: K[A,A] | K[B,A] | K[A,B] | K[B,B] | I128
    kmI = np.zeros((128, 5 * 128), f16)
    kmI[:, 0:128] = km[0:128, 0:128].astype(f16)
    kmI[:, 128:256] = km[128:256, 0:128].astype(f16)
    kmI[:, 256:384] = km[0:128, 128:256].astype(f16)
    kmI[:, 384:512] = km[128:256, 128:256].astype(f16)
    kmI[:, 512:640] = np.eye(128, dtype=f16)
    b5 = np.zeros((47, ROWS), f32)
    for m in range(ROWS):
        b5[m : m + 5, m] = 1.0
    cons = np.zeros((128, 4), f32)
    cons[:, 0] = np.arange(0, 128)      # binsA
    cons[:, 1] = np.arange(128, 256)    # binsB
    cons[:, 2] = 1e-8                   # eps (Ln bias)
    return {"kmI": kmI, "b5": b5, "cons": cons}


def _chunks_for_block(a, b):
    """Split stage-C cols [74a, 74b) into <=CHUNK chunks."""
    lo, hi = HP * a, HP * b
    out = []
    n = hi - lo
    k = (n + CHUNK - 1) // CHUNK
    step = (n + k - 1) // k
    off = lo
    while off < hi:
        cw = min(step, hi - off)
        out.append((off, cw))
        off += cw
    return out


def _build_nc():
    nc = bacc.Bacc("TRN2", target_bir_lowering=False, debug=False)

    xs_d = nc.dram_tensor("xs", [47, 80], dt.float32, kind="ExternalInput")
    xm_d = nc.dram_tensor("xm", [ROWS, 80], dt.float32, kind="ExternalInput")
    b5_d = nc.dram_tensor("b5", [47, ROWS], dt.float32, kind="ExternalInput")
    kmI_d = nc.dram_tensor("kmI", [128, 640], dt.float16, kind="ExternalInput")
    cons_d = nc.dram_tensor("cons", [128, 4], dt.float32, kind="ExternalInput")
    ent_d = nc.dram_tensor("ent", [NP_], dt.float32, kind="ExternalOutput")

    with tile.TileContext(nc) as tc:
        with (
            tc.tile_pool(name="sb", bufs=1) as sb,
            tc.tile_pool(name="lp", bufs=4) as lpp,
            tc.tile_pool(name="mm", bufs=4) as mmp,
            tc.tile_pool(name="psum", bufs=6, space="PSUM") as psum,
            tc.tile_pool(name="psum1", bufs=2, space="PSUM") as psum1,
        ):
            # -------- head: input DMAs on the earliest-starting queues ------
            xt = sb.tile([47, 84], dt.float32)
            nc.vector.memset(xt[:], 0.0)
            b5t = sb.tile([47, ROWS], dt.float32)
            nc.gpsimd.dma_start(b5t[:], b5_d[:])          # Pool queue starts first
            nc.gpsimd.dma_start(xt[:, 2:82], xs_d[:])     # Pool queue
            xm = sb.tile([ROWS, 80], dt.float32)
            nc.sync.dma_start(xm[:], xm_d[:])             # SP queue
            cons = sb.tile([128, 4], dt.float32)
            nc.sync.dma_start(cons[:], cons_d[:])         # SP queue
            km = sb.tile([128, 640], dt.float16)
            nc.scalar.dma_start(km[:], kmI_d[:])          # ACT queue
            binsA = cons[:, 0:1]
            binsB = cons[:, 1:2]
            eps_t = cons[:, 2:3]

            onesrow = sb.tile([1, 128], dt.float16)
            nc.vector.memset(onesrow[:], 1.0)
            ones = sb.tile([128, 1], dt.float16)
            nc.vector.memset(ones[:], 1.0)
            warm = sb.tile([128, 1], dt.float32)
            nc.vector.memset(warm[:], 1.0)
            wout = sb.tile([128, 1], dt.float32)
            # Ln act-table warm-up while ACT is idle (table load ~1.3us)
            nc.scalar.activation(wout[:], warm[:], Act.Ln, bias=1.0)

            # -------- stage A: unsharp preprocessing, col-half ping-pong ----
            sv_ps = psum1.tile([ROWS, 84], dt.float32, tag="mps")
            nc.tensor.matmul(sv_ps[:], b5t[:], xt[:], start=True, stop=True)
            sv = sb.tile([ROWS, 84], dt.float32)
            nc.scalar.copy(sv[:], sv_ps[:])

            t1 = sb.tile([ROWS, 83], dt.float32)
            for c0, c1 in ((0, 44), (44, 83)):
                nc.vector.tensor_add(
                    t1[:, c0:c1], sv_ps[:, c0:c1], sv[:, c0 + 1 : c1 + 1]
                )
            t2 = sb.tile([ROWS, 81], dt.float32)
            for c0, c1 in ((0, 42), (42, 81)):
                nc.vector.tensor_add(
                    t2[:, c0:c1], t1[:, c0:c1], t1[:, c0 + 2 : c1 + 2]
                )
            s25 = sb.tile([ROWS, 80], dt.float32)
            for c0, c1 in ((0, 40), (40, 80)):
                nc.vector.tensor_add(
                    s25[:, c0:c1], t2[:, c0:c1], sv_ps[:, c0 + 4 : c1 + 4]
                )

            def pp(fn):
                for c0, c1 in ((0, 40), (40, 80)):
                    fn(slice(c0, c1))

            tt = sb.tile([ROWS, 80], dt.float32)
            pp(lambda s_: nc.vector.tensor_scalar(
                tt[:, s_], s25[:, s_], INV25, MAGIC, Alu.mult, Alu.add))
            smooth = sb.tile([ROWS, 80], dt.float32)
            pp(lambda s_: nc.vector.tensor_scalar(
                smooth[:, s_], tt[:, s_], MAGIC, None, Alu.subtract))

            denom = sb.tile([ROWS, 80], dt.float32)
            pp(lambda s_: nc.vector.tensor_scalar(
                denom[:, s_], smooth[:, s_], 1e-8, None, Alu.add))
            rr = sb.tile([ROWS, 80], dt.float32)
            pp(lambda s_: nc.vector.reciprocal(rr[:, s_], denom[:, s_]))

            sp = sb.tile([ROWS, 80], dt.float32)
            pp(lambda s_: nc.vector.scalar_tensor_tensor(
                sp[:, s_], smooth[:, s_], -1.25, xm[:, s_], Alu.mult, Alu.add))
            pp(lambda s_: nc.vector.tensor_scalar(
                sp[:, s_], sp[:, s_], 0.0, 255.0, Alu.max, Alu.min))
            pp(lambda s_: nc.vector.tensor_scalar(
                tt[:, s_], sp[:, s_], MAGIC, None, Alu.add))
            sharp = sb.tile([ROWS, 80], dt.float32)
            pp(lambda s_: nc.vector.tensor_scalar(
                sharp[:, s_], tt[:, s_], MAGIC, None, Alu.subtract))

            vv = sb.tile([ROWS, 80], dt.float32)
            pp(lambda s_: nc.vector.scalar_tensor_tensor(
                vv[:, s_], sharp[:, s_], 255.0, rr[:, s_], Alu.mult, Alu.mult))
            pp(lambda s_: nc.vector.tensor_scalar(
                vv[:, s_], vv[:, s_], MAGIC, None, Alu.add))
            dv = sb.tile([ROWS, 80], dt.float16)
            pp(lambda s_: nc.vector.tensor_scalar(
                dv[:, s_], vv[:, s_], MAGIC, 255.0, Alu.subtract, Alu.min))

            dvrow = sb.tile([1, NPIX], dt.float16)
            nc.sync.dma_start(dvrow[:], dv[:])            # SP queue

            # -------- broadcast + one-hot + trees + stage C -----------------
            dv_bc = sb.tile([128, NPIX], dt.float16, tag="dv_bc")
            oh = sb.tile([128, 2 * NPIX], dt.float16, tag="oh")
            bc_chunks = []
            boff = 0
            while boff < NPIX:
                bw = min(BCCH, NPIX - boff)
                bc_chunks.append((boff, bw))
                boff += bw

            bc_ps0 = None
            for k, (off, bw) in enumerate(bc_chunks):
                bc_ps = psum.tile([128, bw], dt.float32, tag="g_ps", name="bc")
                nc.tensor.matmul(
                    bc_ps[:], onesrow[:], dvrow[:, off : off + bw],
                    start=True, stop=True,
                )
                if k == 0:
                    bc_ps0 = bc_ps   # chunk 0 is_eq reads PSUM directly
                else:
                    nc.scalar.copy(dv_bc[:, off : off + bw], bc_ps[:])

            def is_eq_group(g0, g1, src=None):
                src = src if src is not None else dv_bc[:, g0:g1]
                nc.vector.tensor_scalar(
                    oh[:, g0:g1], src, binsA, None, Alu.is_equal
                )
                nc.vector.tensor_scalar(
                    oh[:, NPIX + g0 : NPIX + g1], src, binsB, None, Alu.is_equal,
                )

            v1 = sb.tile([128, 2 * 41 * 80], dt.float16, tag="v1")
            v2 = sb.tile([128, 2 * 39 * 80], dt.float16, tag="v2")
            u2 = sb.tile([128, 2 * PR * 80], dt.float16, tag="u2")
            v7 = sb.tile([128, 2 * PR * 80], dt.float16, tag="v7")
            t1h = sb.tile([128, 2 * PR * 79], dt.float16, tag="t1h")
            t2h = sb.tile([128, 2 * PR * 77], dt.float16, tag="t2h")
            uh = sb.tile([128, 2 * PR * HP], dt.float16, tag="uh")
            h_f = sb.tile([128, 2 * NP_], dt.float16, tag="h_f")

            oh4 = oh[:].rearrange("p (h r c) -> p h r c", h=2, r=ROWS, c=80)
            v1v = v1[:].rearrange("p (h r c) -> p h r c", h=2, r=41, c=80)
            v2v = v2[:].rearrange("p (h r c) -> p h r c", h=2, r=39, c=80)
            u2v = u2[:].rearrange("p (h r c) -> p h r c", h=2, r=PR, c=80)
            v7v = v7[:].rearrange("p (h r c) -> p h r c", h=2, r=PR, c=80)
            t1v = t1h[:].rearrange("p (h r c) -> p h r c", h=2, r=PR, c=79)
            t2v = t2h[:].rearrange("p (h r c) -> p h r c", h=2, r=PR, c=77)
            uhv = uh[:].rearrange("p (h r c) -> p h r c", h=2, r=PR, c=HP)
            hfv = h_f[:].rearrange("p (h r c) -> p h r c", h=2, r=PR, c=HP)

            def v_level(dstv, s1v, sh1, s2v, sh2, lo, hi):
                # one op per bin-half: contiguous slices, consecutive ops
                # independent (ping-pong) so the DVE never stalls on acks
                for h in (slice(0, 1), slice(1, 2)):
                    nc.vector.tensor_add(
                        dstv[:, h, lo:hi, :],
                        s1v[:, h, lo + sh1 : hi + sh1, :],
                        s2v[:, h, lo + sh2 : hi + sh2, :],
                    )

            def v_block(a, b):
                a1, b1 = (0 if a == 0 else a + 4), min(b + 4, 41)
                a2, b2 = (0 if a == 0 else a + 2), min(b + 2, 39)
                v_level(v1v, oh4, 0, oh4, 1, a1, b1)
                v_level(v2v, v1v, 0, v1v, 2, a2, b2)
                v_level(u2v, v2v, 0, v1v, 4, a, b)
                v_level(v7v, u2v, 0, oh4, 6, a, b)

            def h_group(a, b):
                for h in (slice(0, 1), slice(1, 2)):
                    nc.vector.tensor_add(
                        t1v[:, h, a:b, :], v7v[:, h, a:b, 0:79],
                        v7v[:, h, a:b, 1:80],
                    )
                for h in (slice(0, 1), slice(1, 2)):
                    nc.vector.tensor_add(
                        t2v[:, h, a:b, :], t1v[:, h, a:b, 0:77],
                        t1v[:, h, a:b, 2:79],
                    )
                for h in (slice(0, 1), slice(1, 2)):
                    nc.vector.tensor_add(
                        uhv[:, h, a:b, :], t2v[:, h, a:b, 0:HP],
                        t1v[:, h, a:b, 4 : 4 + HP],
                    )
                for h in (slice(0, 1), slice(1, 2)):
                    nc.vector.tensor_add(
                        hfv[:, h, a:b, :], uhv[:, h, a:b, :],
                        v7v[:, h, a:b, 6:80],
                    )

            def stage_c_pe_act(off, cw):
                h0c = h_f[:, off : off + cw]
                h1c = h_f[:, NP_ + off : NP_ + off + cw]
                g0 = psum.tile([128, cw], dt.float32, tag="g_ps", name="g0")
                nc.tensor.matmul(g0[:], km[:, 0:128], h0c, start=True, stop=False)
                nc.tensor.matmul(g0[:], km[:, 128:256], h1c, start=False, stop=True)
                g1 = psum.tile([128, cw], dt.float32, tag="g_ps", name="g1")
                nc.tensor.matmul(g1[:], km[:, 256:384], h0c, start=True, stop=False)
                nc.tensor.matmul(g1[:], km[:, 384:512], h1c, start=False, stop=True)
                lp0 = lpp.tile([128, cw], dt.float16, tag="lp0", name="lp0")
                nc.scalar.activation(lp0[:], g0[:], Act.Ln, bias=eps_t, scale=LN_SCALE)
                lp1 = lpp.tile([128, cw], dt.float16, tag="lp1", name="lp1")
                nc.scalar.activation(lp1[:], g1[:], Act.Ln, bias=eps_t, scale=LN_SCALE)
                return lp0, lp1

            def stage_c_muls(off, cw, lp0, lp1):
                h0c = h_f[:, off : off + cw]
                h1c = h_f[:, NP_ + off : NP_ + off + cw]
                m0 = mmp.tile([128, cw], dt.float16, tag="m0", name="m0")
                nc.vector.tensor_mul(m0[:], h0c, lp0[:])
                m1 = mmp.tile([128, cw], dt.float16, tag="m1", name="m1")
                nc.vector.tensor_mul(m1[:], h1c, lp1[:])
                return m0, m1

            def stage_c_out(off, cw, m0, m1):
                e_ps = psum1.tile([1, cw], dt.float32, tag="mps")
                nc.tensor.matmul(e_ps[:], ones[:], m0[:], start=True, stop=False)
                nc.tensor.matmul(e_ps[:], ones[:], m1[:], start=False, stop=True)
                ent_c = lpp.tile([1, cw], dt.float32, tag="entc", name="entc")
                nc.scalar.activation(
                    ent_c[:], e_ps[:], Act.Copy, bias=0.0, scale=NEG_INV_L
                )
                nc.sync.dma_start(ent_d[off : off + cw], ent_c[:])

            # ---- interleaved emission ----
            chunks = [[(HP * a, HP * (b - a)) for a, b in grp] for grp in C_ROWS]

            is_eq_group(0, bc_chunks[0][1], src=bc_ps0[:])
            is_eq_group(bc_chunks[0][1], OH_GROUPS[1])
            is_eq_group(OH_GROUPS[1], OH_GROUPS[2])
            v_block(*V_BLOCKS[0])
            is_eq_group(OH_GROUPS[2], OH_GROUPS[3])
            h_group(*H_GROUPS[0])
            lps0 = [stage_c_pe_act(off, cw) for off, cw in chunks[0]]
            v_block(*V_BLOCKS[1])
            ms0 = [stage_c_muls(off, cw, *lp)
                   for (off, cw), lp in zip(chunks[0], lps0)]
            h_group(*H_GROUPS[1])
            for (off, cw), m in zip(chunks[0], ms0):
                stage_c_out(off, cw, *m)
            lps1 = [stage_c_pe_act(off, cw) for off, cw in chunks[1]]
            ms1 = [stage_c_muls(off, cw, *lp)
                   for (off, cw), lp in zip(chunks[1], lps1)]
            h_group(*H_GROUPS[2])
            for (off, cw), m in zip(chunks[1], ms1):
                stage_c_out(off, cw, *m)
            lps2 = [stage_c_pe_act(off, cw) for off, cw in chunks[2]]
            ms2 = [stage_c_muls(off, cw, *lp)
                   for (off, cw), lp in zip(chunks[2], lps2)]
            for (off, cw), m in zip(chunks[2], ms2):
                stage_c_out(off, cw, *m)

    nc.compile()
    return nc


def _get_compiled():
    global _COMPILED
    if _COMPILED is None:
        _COMPILED = (_build_nc(), _host_constants())
    return _COMPILED


def _in_map_for_core(xi, core, consts):
    b, half = core // 2, core % 2
    r0 = half * PR
    strip = np.zeros((47, 80), f32)
    lo, hi = r0 - 2, r0 + 45
    slo, shi = max(lo, 0), min(hi, 80)
    strip[slo - lo : shi - lo] = xi[b, slo:shi]
    m = dict(consts)
    m["xs"] = strip
    m["xm"] = np.ascontiguousarray(f32(2.5) * strip[2 : 2 + ROWS])
    return m


def _run(x, trace=False, **kw):
    """x: (2,2,1,80,80) float32. Returns BassKernelResults."""
    nc, consts = _get_compiled()
    xi = np.ascontiguousarray(np.asarray(x, f32).reshape(4, 80, 80))
    in_maps = [_in_map_for_core(xi, core, consts) for core in range(8)]
    res = run_bass_kernel_spmd(nc, in_maps, list(range(8)), trace=trace, **kw)
    return res


def kernel(x):
    res = _run(x)
    out = np.zeros((4, 80, 80), f32)
    pad = R // 2
    for core in range(8):
        b, half = core // 2, core % 2
        r0 = half * PR
        ent = np.asarray(res.results[core]["ent"], f32).reshape(PR, HP)
        out[b, pad + r0 : pad + r0 + PR, pad : pad + HP] = ent
    return out.reshape(2, 2, 80, 80)


# revision 22
# speedup vs baseline: 1.1688x; 1.1688x over previous
"""Trainium2 Bass kernel for nn_Entropy (KDE local-entropy via histogram binning).

Contract: kernel(**inputs) takes the FULL input x (2,2,1,80,80) fp32 and
returns the FULL output (2,2,80,80) fp32, sharding internally across 8
NeuronCores (core = batch*2 + row-half of the 74x74 patch grid).

Algorithm (per core, one 47x80 input strip -> 37x74 entropy block):
  1. unsharp preprocessing (5x5 blur via PE banded matmul + free-axis tree
     adds, exact-tie-aware rounding, IEEE-reciprocal division) -> integer
     "division" image in [0,255].
  2. KDE entropy: per 7x7 patch the pairwise-exp sum collapses onto a
     256-bin histogram h; G = K @ h with the constant 256x256 kernel matrix
     K[b,b'] = exp(-(b-b')^2/12.5); ent = -(1/49) * h . log(G/(49*norm)+eps).
     h is built from a fused fp16 one-hot image (bins on partitions, both
     128-bin halves side by side) box-summed with shifted-add trees
     (7 = 4+2+1) vertically then horizontally; all counts are fp16-exact.

Pipelining layout (v2): input DMAs ride separate engine queues; the one-hot
is_equal, the vertical/horizontal shifted-add trees and stage C all run in
row-blocks so PE (G matmuls), ACT (Ln/copies) and DVE (adds/muls) overlap;
the -1/49 entropy scale is folded into the PSUM->SBUF output copy on ACT.
"""
import os
import sys

import numpy as np

for _p in ("/opt/trn_rl_repo", "/root/.axon_site/_ro/trn_rl_repo"):
    if os.path.isdir(_p) and _p not in sys.path:
        sys.path.insert(0, _p)

import concourse.bass as bass
import concourse.bacc as bacc
import concourse.tile as tile
from concourse import mybir
from concourse.bass_utils import run_bass_kernel_spmd

dt = mybir.dt
Alu = mybir.AluOpType
Act = mybir.ActivationFunctionType
f32 = np.float32

R = 7
BW = 2.5
L = R * R  # 49
NORM = f32((2.0 * np.pi * BW * BW) ** 0.5)  # C=1 -> exponent 1/2
LN_SCALE = float(f32(1.0 / (L * NORM)))
INV25 = float(f32(1.0) / f32(25.0))
NEG_INV_L = float(-(f32(1.0) / f32(L)))
MAGIC = 8388608.0  # RNE(v) == (v + 2^23) - 2^23 for 0 <= v < 2^23

# geometry
HP = 74          # patch grid cols (80 - 7 + 1)
ROWS = 43        # division-image rows needed per core (37 patch rows + 6)
PR = 37          # patch rows per core
NPIX = ROWS * 80         # 3440
NP_ = PR * HP            # 2738

# schedule knobs
BCCH = 492                           # broadcast chunk (<=512 psum cols)
OH_GROUPS = [0, 1000, 2000, NPIX]    # is_equal spans (pixel index)
V_BLOCKS = [(0, 18), (18, PR)]       # vertical-tree row blocks
H_GROUPS = [(0, 12), (12, 24), (24, PR)]   # h-tree / stage-C row groups
C_ROWS = [[(0, 6), (6, 12)], [(12, 18), (18, 24)], [(24, 30), (30, 36), (36, PR)]]

_COMPILED = None


def _host_constants():
    f16 = np.float16
    bins = np.arange(256, dtype=np.float64)
    km = np.exp(-((bins[:, None] - bins[None, :]) ** 2) / (2.0 * BW * BW))
    # kmI layout: [128, 5*128]: K[A,A] | K[B,A] | K[A,B] | K[B,B] | I128
    kmI = np.zeros((128, 5 * 128), f16)
    kmI[:, 0:128] = km[0:128, 0:128].astype(f16)
    kmI[:, 128:256] = km[128:256, 0:128].astype(f16)
    kmI[:, 256:384] = km[0:128, 128:256].astype(f16)
    kmI[:, 384:512] = km[128:256, 128:256].astype(f16)
    kmI[:, 512:640] = np.eye(128, dtype=f16)
    b5 = np.zeros((47, ROWS), f32)
    for m in range(ROWS):
        b5[m : m + 5, m] = 1.0
    cons = np.zeros((128, 4), f32)
    cons[:, 0] = np.arange(0, 128)      # binsA
    cons[:, 1] = np.arange(128, 256)    # binsB
    cons[:, 2] = 1e-8                   # eps (Ln bias)
    return {"kmI": kmI, "b5": b5, "cons": cons}


def _chunks_for_block(a, b):
    """Split stage-C cols [74a, 74b) into <=CHUNK chunks."""
    lo, hi = HP * a, HP * b
    out = []
    n = hi - lo
    k = (n + CHUNK - 1) // CHUNK
    step = (n + k - 1) // k
    off = lo
    while off < hi:
        cw = min(step, hi - off)
        out.append((off, cw))
        off += cw
    return out


def _build_nc():
    nc = bacc.Bacc("TRN2", target_bir_lowering=False, debug=False)

    xs_d = nc.dram_tensor("xs", [47, 80], dt.float32, kind="ExternalInput")
    xm_d = nc.dram_tensor("xm", [ROWS, 80], dt.float32, kind="ExternalInput")
    b5_d = nc.dram_tensor("b5", [47, ROWS], dt.float32, kind="ExternalInput")
    kmI_d = nc.dram_tensor("kmI", [128, 640], dt.float16, kind="ExternalInput")
    cons_d = nc.dram_tensor("cons", [128, 4], dt.float32, kind="ExternalInput")
    ent_d = nc.dram_tensor("ent", [NP_], dt.float32, kind="ExternalOutput")

    with tile.TileContext(nc) as tc:
        with (
            tc.tile_pool(name="sb", bufs=1) as sb,
            tc.tile_pool(name="lp", bufs=4) as lpp,
            tc.tile_pool(name="mm", bufs=4) as mmp,
            tc.tile_pool(name="psum", bufs=6, space="PSUM") as psum,
            tc.tile_pool(name="psum1", bufs=2, space="PSUM") as psum1,
        ):
            # -------- head: input DMAs on the earliest-starting queues ------
            xt = sb.tile([47, 84], dt.float32)
            nc.vector.memset(xt[:], 0.0)
            nc.sync.dma_start(xt[:, 2:82], xs_d[:])       # SP queue (first)
            b5t = sb.tile([47, ROWS], dt.float32)
            nc.sync.dma_start(b5t[:], b5_d[:])            # SP queue
            xm = sb.tile([ROWS, 80], dt.float32)
            nc.gpsimd.dma_start(xm[:], xm_d[:])           # Pool queue
            cons = sb.tile([128, 4], dt.float32)
            nc.gpsimd.dma_start(cons[:], cons_d[:])       # Pool queue
            km = sb.tile([128, 640], dt.float16)
            nc.scalar.dma_start(km[:], kmI_d[:])          # ACT queue
            binsA = cons[:, 0:1]
            binsB = cons[:, 1:2]
            eps_t = cons[:, 2:3]

            onesrow = sb.tile([1, 128], dt.float16)
            nc.vector.memset(onesrow[:], 1.0)
            ones = sb.tile([128, 1], dt.float16)
            nc.vector.memset(ones[:], 1.0)
            warm = sb.tile([128, 1], dt.float32)
            nc.vector.memset(warm[:], 1.0)
            wout = sb.tile([128, 1], dt.float32)
            # Ln act-table warm-up while ACT is idle (table load ~1.3us)
            nc.scalar.activation(wout[:], warm[:], Act.Ln, bias=1.0)

            # -------- stage A: unsharp preprocessing, col-half ping-pong ----
            sv_ps = psum1.tile([ROWS, 84], dt.float32, tag="mps")
            nc.tensor.matmul(sv_ps[:], b5t[:], xt[:], start=True, stop=True)
            sv = sb.tile([ROWS, 84], dt.float32)
            nc.scalar.copy(sv[:], sv_ps[:])

            t1 = sb.tile([ROWS, 83], dt.float32)
            for c0, c1 in ((0, 44), (44, 83)):
                nc.vector.tensor_add(
                    t1[:, c0:c1], sv_ps[:, c0:c1], sv[:, c0 + 1 : c1 + 1]
                )
            t2 = sb.tile([ROWS, 81], dt.float32)
            for c0, c1 in ((0, 42), (42, 81)):
                nc.vector.tensor_add(
                    t2[:, c0:c1], t1[:, c0:c1], t1[:, c0 + 2 : c1 + 2]
                )
            s25 = sb.tile([ROWS, 80], dt.float32)
            for c0, c1 in ((0, 40), (40, 80)):
                nc.vector.tensor_add(
                    s25[:, c0:c1], t2[:, c0:c1], sv_ps[:, c0 + 4 : c1 + 4]
                )

            def pp(fn):
                for c0, c1 in ((0, 40), (40, 80)):
                    fn(slice(c0, c1))

            tt = sb.tile([ROWS, 80], dt.float32)
            pp(lambda s_: nc.vector.tensor_scalar(
                tt[:, s_], s25[:, s_], INV25, MAGIC, Alu.mult, Alu.add))
            smooth = sb.tile([ROWS, 80], dt.float32)
            pp(lambda s_: nc.vector.tensor_scalar(
                smooth[:, s_], tt[:, s_], MAGIC, None, Alu.subtract))

            denom = sb.tile([ROWS, 80], dt.float32)
            pp(lambda s_: nc.vector.tensor_scalar(
                denom[:, s_], smooth[:, s_], 1e-8, None, Alu.add))
            rr = sb.tile([ROWS, 80], dt.float32)
            pp(lambda s_: nc.vector.reciprocal(rr[:, s_], denom[:, s_]))

            sp = sb.tile([ROWS, 80], dt.float32)
            pp(lambda s_: nc.vector.scalar_tensor_tensor(
                sp[:, s_], smooth[:, s_], -1.25, xm[:, s_], Alu.mult, Alu.add))
            pp(lambda s_: nc.vector.tensor_scalar(
                sp[:, s_], sp[:, s_], 0.0, 255.0, Alu.max, Alu.min))
            pp(lambda s_: nc.vector.tensor_scalar(
                tt[:, s_], sp[:, s_], MAGIC, None, Alu.add))
            sharp = sb.tile([ROWS, 80], dt.float32)
            pp(lambda s_: nc.vector.tensor_scalar(
                sharp[:, s_], tt[:, s_], MAGIC, None, Alu.subtract))

            vv = sb.tile([ROWS, 80], dt.float32)
            pp(lambda s_: nc.vector.scalar_tensor_tensor(
                vv[:, s_], sharp[:, s_], 255.0, rr[:, s_], Alu.mult, Alu.mult))
            pp(lambda s_: nc.vector.tensor_scalar(
                vv[:, s_], vv[:, s_], MAGIC, None, Alu.add))
            dv = sb.tile([ROWS, 80], dt.float16)
            pp(lambda s_: nc.vector.tensor_scalar(
                dv[:, s_], vv[:, s_], MAGIC, 255.0, Alu.subtract, Alu.min))

            dvrow = sb.tile([1, NPIX], dt.float16)
            nc.sync.dma_start(dvrow[:], dv[:])            # SP queue

            # -------- broadcast + one-hot + trees + stage C -----------------
            dv_bc = sb.tile([128, NPIX], dt.float16, tag="dv_bc")
            oh = sb.tile([128, 2 * NPIX], dt.float16, tag="oh")
            bc_chunks = []
            boff = 0
            while boff < NPIX:
                bw = min(BCCH, NPIX - boff)
                bc_chunks.append((boff, bw))
                boff += bw

            bc_ps0 = None
            for k, (off, bw) in enumerate(bc_chunks):
                bc_ps = psum.tile([128, bw], dt.float32, tag="g_ps", name="bc")
                nc.tensor.matmul(
                    bc_ps[:], onesrow[:], dvrow[:, off : off + bw],
                    start=True, stop=True,
                )
                if k == 0:
                    bc_ps0 = bc_ps   # chunk 0 is_eq reads PSUM directly
                else:
                    nc.scalar.copy(dv_bc[:, off : off + bw], bc_ps[:])

            def is_eq_group(g0, g1, src=None):
                src = src if src is not None else dv_bc[:, g0:g1]
                nc.vector.tensor_scalar(
                    oh[:, g0:g1], src, binsA, None, Alu.is_equal
                )
                nc.vector.tensor_scalar(
                    oh[:, NPIX + g0 : NPIX + g1], src, binsB, None, Alu.is_equal,
                )

            v1 = sb.tile([128, 2 * 41 * 80], dt.float16, tag="v1")
            v2 = sb.tile([128, 2 * 39 * 80], dt.float16, tag="v2")
            u2 = sb.tile([128, 2 * PR * 80], dt.float16, tag="u2")
            v7 = sb.tile([128, 2 * PR * 80], dt.float16, tag="v7")
            t1h = sb.tile([128, 2 * PR * 79], dt.float16, tag="t1h")
            t2h = sb.tile([128, 2 * PR * 77], dt.float16, tag="t2h")
            uh = sb.tile([128, 2 * PR * HP], dt.float16, tag="uh")
            h_f = sb.tile([128, 2 * NP_], dt.float16, tag="h_f")

            oh4 = oh[:].rearrange("p (h r c) -> p h r c", h=2, r=ROWS, c=80)
            v1v = v1[:].rearrange("p (h r c) -> p h r c", h=2, r=41, c=80)
            v2v = v2[:].rearrange("p (h r c) -> p h r c", h=2, r=39, c=80)
            u2v = u2[:].rearrange("p (h r c) -> p h r c", h=2, r=PR, c=80)
            v7v = v7[:].rearrange("p (h r c) -> p h r c", h=2, r=PR, c=80)
            t1v = t1h[:].rearrange("p (h r c) -> p h r c", h=2, r=PR, c=79)
            t2v = t2h[:].rearrange("p (h r c) -> p h r c", h=2, r=PR, c=77)
            uhv = uh[:].rearrange("p (h r c) -> p h r c", h=2, r=PR, c=HP)
            hfv = h_f[:].rearrange("p (h r c) -> p h r c", h=2, r=PR, c=HP)

            def v_level(dstv, s1v, sh1, s2v, sh2, lo, hi):
                # one op per bin-half: contiguous slices, consecutive ops
                # independent (ping-pong) so the DVE never stalls on acks
                for h in (slice(0, 1), slice(1, 2)):
                    nc.vector.tensor_add(
                        dstv[:, h, lo:hi, :],
                        s1v[:, h, lo + sh1 : hi + sh1, :],
                        s2v[:, h, lo + sh2 : hi + sh2, :],
                    )

            def v_block(a, b, first=False):
                a1, b1 = (a if first else a + 4), min(b + 4, 41)
                a2, b2 = (a if first else a + 2), min(b + 2, 39)
                v_level(v1v, oh4, 0, oh4, 1, a1, b1)
                v_level(v2v, v1v, 0, v1v, 2, a2, b2)
                v_level(u2v, v2v, 0, v1v, 4, a, b)
                v_level(v7v, u2v, 0, oh4, 6, a, b)

            def h_group(a, b):
                for h in (slice(0, 1), slice(1, 2)):
                    nc.vector.tensor_add(
                        t1v[:, h, a:b, :], v7v[:, h, a:b, 0:79],
                        v7v[:, h, a:b, 1:80],
                    )
                for h in (slice(0, 1), slice(1, 2)):
                    nc.vector.tensor_add(
                        t2v[:, h, a:b, :], t1v[:, h, a:b, 0:77],
                        t1v[:, h, a:b, 2:79],
                    )
                for h in (slice(0, 1), slice(1, 2)):
                    nc.vector.tensor_add(
                        uhv[:, h, a:b, :], t2v[:, h, a:b, 0:HP],
                        t1v[:, h, a:b, 4 : 4 + HP],
                    )
                for h in (slice(0, 1), slice(1, 2)):
                    nc.vector.tensor_add(
                        hfv[:, h, a:b, :], uhv[:, h, a:b, :],
                        v7v[:, h, a:b, 6:80],
                    )

            def stage_c_pe_act(off, cw):
                h0c = h_f[:, off : off + cw]
                h1c = h_f[:, NP_ + off : NP_ + off + cw]
                g0 = psum.tile([128, cw], dt.float32, tag="g_ps", name="g0")
                nc.tensor.matmul(g0[:], km[:, 0:128], h0c, start=True, stop=False)
                nc.tensor.matmul(g0[:], km[:, 128:256], h1c, start=False, stop=True)
                g1 = psum.tile([128, cw], dt.float32, tag="g_ps", name="g1")
                nc.tensor.matmul(g1[:], km[:, 256:384], h0c, start=True, stop=False)
                nc.tensor.matmul(g1[:], km[:, 384:512], h1c, start=False, stop=True)
                lp0 = lpp.tile([128, cw], dt.float16, tag="lp0", name="lp0")
                nc.scalar.activation(lp0[:], g0[:], Act.Ln, bias=eps_t, scale=LN_SCALE)
                lp1 = lpp.tile([128, cw], dt.float16, tag="lp1", name="lp1")
                nc.scalar.activation(lp1[:], g1[:], Act.Ln, bias=eps_t, scale=LN_SCALE)
                return lp0, lp1

            def stage_c_muls(off, cw, lp0, lp1):
                h0c = h_f[:, off : off + cw]
                h1c = h_f[:, NP_ + off : NP_ + off + cw]
                m0 = mmp.tile([128, cw], dt.float16, tag="m0", name="m0")
                nc.vector.tensor_mul(m0[:], h0c, lp0[:])
                m1 = mmp.tile([128, cw], dt.float16, tag="m1", name="m1")
                nc.vector.tensor_mul(m1[:], h1c, lp1[:])
                return m0, m1

            def stage_c_out(off, cw, m0, m1):
                e_ps = psum1.tile([1, cw], dt.float32, tag="mps")
                nc.tensor.matmul(e_ps[:], ones[:], m0[:], start=True, stop=False)
                nc.tensor.matmul(e_ps[:], ones[:], m1[:], start=False, stop=True)
                ent_c = lpp.tile([1, cw], dt.float32, tag="entc", name="entc")
                nc.scalar.activation(
                    ent_c[:], e_ps[:], Act.Copy, bias=0.0, scale=NEG_INV_L
                )
                nc.sync.dma_start(ent_d[off : off + cw], ent_c[:])

            # ---- interleaved emission ----
            chunks = [(HP * a, HP * (b - a)) for a, b in C_ROWS]

            is_eq_group(0, bc_chunks[0][1], src=bc_ps0[:])
            is_eq_group(bc_chunks[0][1], OH_GROUPS[1])
            is_eq_group(OH_GROUPS[1], OH_GROUPS[2])
            v_block(*V_BLOCKS[0], first=True)
            is_eq_group(OH_GROUPS[2], OH_GROUPS[3])
            # keep the PE continuously busy while the v-tree runs so it holds
            # max pstate when the tap lane starts (idle resets the clock ramp)
            warm_ps = psum1.tile([1, 512], dt.float32, tag="mps", name="warm")
            for _ in range(14):
                nc.tensor.matmul(
                    warm_ps[:], ones[:], oh[0:128, 0:512], start=True, stop=True
                )
            for r0 in range(0, 12, TAP_BATCH):        # PE h-taps rows [0,12)
                h_taps(r0, TAP_BATCH)
            lps_a = [stage_c_pe_act(off, cw) for off, cw in chunks[0:2]]
            v_block(*V_BLOCKS[1])
            for r0 in range(12, PE_ROWS, TAP_BATCH):  # PE h-taps rows [12,24)
                h_taps(r0, TAP_BATCH)
            lps_b = [stage_c_pe_act(off, cw) for off, cw in chunks[2:4]]
            v_block(*V_BLOCKS[2])
            ms_a = [stage_c_muls(off, cw, lp)
                    for (off, cw), lp in zip(chunks[0:2], lps_a)]
            h_group(PE_ROWS, 34)                      # DVE h rows [24,34)
            v_block(*V_BLOCKS[3])
            ms_b = [stage_c_muls(off, cw, lp)
                    for (off, cw), lp in zip(chunks[2:4], lps_b)]
            lps_c = [stage_c_pe_act(off, cw) for off, cw in chunks[4:6]]
            h_group(34, PR)                           # DVE h rows [34,37)
            lps_d = [stage_c_pe_act(off, cw) for off, cw in chunks[6:]]
            for (off, cw), m in zip(chunks[0:2], ms_a):
                stage_c_out(off, cw, m)
            for (off, cw), m in zip(chunks[2:4], ms_b):
                stage_c_out(off, cw, m)
            ms_c = [stage_c_muls(off, cw, lp)
                    for (off, cw), lp in zip(chunks[4:6], lps_c)]
            ms_d = [stage_c_muls(off, cw, lp)
                    for (off, cw), lp in zip(chunks[6:], lps_d)]
            for (off, cw), m in zip(chunks[4:6], ms_c):
                stage_c_out(off, cw, m)
            for (off, cw), m in zip(chunks[6:], ms_d):
                stage_c_out(off, cw, m)

    nc.compile()
    return nc


def _get_compiled():
    global _COMPILED
    if _COMPILED is None:
        _COMPILED = (_build_nc(), _host_constants())
    return _COMPILED


def _in_map_for_core(xi, core, consts):
    b, half = core // 2, core % 2
    r0 = half * PR
    strip = np.zeros((47, 80), f32)
    lo, hi = r0 - 2, r0 + 45
    slo, shi = max(lo, 0), min(hi, 80)
    strip[slo - lo : shi - lo] = xi[b, slo:shi]
    m = dict(consts)
    m["xs"] = strip
    m["xm"] = np.ascontiguousarray(f32(2.5) * strip[2 : 2 + ROWS])
    return m


def _run(x, trace=False, **kw):
    """x: (2,2,1,80,80) float32. Returns BassKernelResults."""
    nc, consts = _get_compiled()
    xi = np.ascontiguousarray(np.asarray(x, f32).reshape(4, 80, 80))
    in_maps = [_in_map_for_core(xi, core, consts) for core in range(8)]
    res = run_bass_kernel_spmd(nc, in_maps, list(range(8)), trace=trace, **kw)
    return res


def kernel(x):
    res = _run(x)
    out = np.zeros((4, 80, 80), f32)
    pad = R // 2
    for core in range(8):
        b, half = core // 2, core % 2
        r0 = half * PR
        ent = np.asarray(res.results[core]["ent"], f32).reshape(PR, HP)
        out[b, pad + r0 : pad + r0 + PR, pad : pad + HP] = ent
    return out.reshape(2, 2, 80, 80)


# revision 24
# speedup vs baseline: 1.1899x; 1.0181x over previous
"""Trainium2 Bass kernel for nn_Entropy (KDE local-entropy via histogram binning).

Contract: kernel(**inputs) takes the FULL input x (2,2,1,80,80) fp32 and
returns the FULL output (2,2,80,80) fp32, sharding internally across 8
NeuronCores (core = batch*2 + row-half of the 74x74 patch grid).

Algorithm (per core, one 47x80 input strip -> 37x74 entropy block):
  1. unsharp preprocessing (5x5 blur via PE banded matmul + free-axis tree
     adds, exact-tie-aware rounding, IEEE-reciprocal division) -> integer
     "division" image in [0,255].
  2. KDE entropy: per 7x7 patch the pairwise-exp sum collapses onto a
     256-bin histogram h; G = K @ h with the constant 256x256 kernel matrix
     K[b,b'] = exp(-(b-b')^2/12.5); ent = -(1/49) * h . log(G/(49*norm)+eps).
     h is built from a fused fp16 one-hot image (bins on partitions, both
     128-bin halves side by side) box-summed with shifted-add trees
     (7 = 4+2+1) vertically then horizontally; all counts are fp16-exact.

Pipelining layout (v2): input DMAs ride separate engine queues; the one-hot
is_equal, the vertical/horizontal shifted-add trees and stage C all run in
row-blocks so PE (G matmuls), ACT (Ln/copies) and DVE (adds/muls) overlap;
the -1/49 entropy scale is folded into the PSUM->SBUF output copy on ACT.
"""
import os
import sys

import numpy as np

for _p in ("/opt/trn_rl_repo", "/root/.axon_site/_ro/trn_rl_repo"):
    if os.path.isdir(_p) and _p not in sys.path:
        sys.path.insert(0, _p)

import concourse.bass as bass
import concourse.bacc as bacc
import concourse.tile as tile
from concourse import mybir
from concourse.bass_utils import run_bass_kernel_spmd

dt = mybir.dt
Alu = mybir.AluOpType
Act = mybir.ActivationFunctionType
f32 = np.float32

R = 7
BW = 2.5
L = R * R  # 49
NORM = f32((2.0 * np.pi * BW * BW) ** 0.5)  # C=1 -> exponent 1/2
LN_SCALE = float(f32(1.0 / (L * NORM)))
INV25 = float(f32(1.0) / f32(25.0))
NEG_INV_L = float(-(f32(1.0) / f32(L)))
MAGIC = 8388608.0  # RNE(v) == (v + 2^23) - 2^23 for 0 <= v < 2^23

# geometry
HP = 74          # patch grid cols (80 - 7 + 1)
ROWS = 43        # division-image rows needed per core (37 patch rows + 6)
PR = 37          # patch rows per core
NPIX = ROWS * 80         # 3440
NP_ = PR * HP            # 2738

# schedule knobs
BCCH = 430                           # broadcast chunk (<=512 psum cols)
OH_GROUPS = [0, 1000, 2000, NPIX]    # is_equal spans (pixel index)
V_BLOCKS = [(0, 18), (18, PR)]       # vertical-tree row blocks
H_GROUPS = [(0, 12), (12, 24), (24, PR)]   # h-tree / stage-C row groups
C_ROWS = [[(0, 6), (6, 12)], [(12, 18), (18, 24)], [(24, 30), (30, 36), (36, PR)]]

_COMPILED = None


def _host_constants():
    f16 = np.float16
    bins = np.arange(256, dtype=np.float64)
    km = np.exp(-((bins[:, None] - bins[None, :]) ** 2) / (2.0 * BW * BW))
    # kmI layout: [128, 5*128]: K[A,A] | K[B,A] | K[A,B] | K[B,B] | I128
    kmI = np.zeros((128, 5 * 128), f16)
    kmI[:, 0:128] = km[0:128, 0:128].astype(f16)
    kmI[:, 128:256] = km[128:256, 0:128].astype(f16)
    kmI[:, 256:384] = km[0:128, 128:256].astype(f16)
    kmI[:, 384:512] = km[128:256, 128:256].astype(f16)
    kmI[:, 512:640] = np.eye(128, dtype=f16)
    b5 = np.zeros((47, ROWS), f32)
    for m in range(ROWS):
        b5[m : m + 5, m] = 1.0
    cons = np.zeros((128, 4), f32)
    cons[:, 0] = np.arange(0, 128)      # binsA
    cons[:, 1] = np.arange(128, 256)    # binsB
    cons[:, 2] = 1e-8                   # eps (Ln bias)
    return {"kmI": kmI, "b5": b5, "cons": cons}


def _chunks_for_block(a, b):
    """Split stage-C cols [74a, 74b) into <=CHUNK chunks."""
    lo, hi = HP * a, HP * b
    out = []
    n = hi - lo
    k = (n + CHUNK - 1) // CHUNK
    step = (n + k - 1) // k
    off = lo
    while off < hi:
        cw = min(step, hi - off)
        out.append((off, cw))
        off += cw
    return out


def _build_nc():
    nc = bacc.Bacc("TRN2", target_bir_lowering=False, debug=False)

    xs_d = nc.dram_tensor("xs", [47, 80], dt.float32, kind="ExternalInput")
    xm_d = nc.dram_tensor("xm", [ROWS, 80], dt.float32, kind="ExternalInput")
    b5_d = nc.dram_tensor("b5", [47, ROWS], dt.float32, kind="ExternalInput")
    kmI_d = nc.dram_tensor("kmI", [128, 640], dt.float16, kind="ExternalInput")
    cons_d = nc.dram_tensor("cons", [128, 4], dt.float32, kind="ExternalInput")
    ent_d = nc.dram_tensor("ent", [NP_], dt.float32, kind="ExternalOutput")

    with tile.TileContext(nc) as tc:
        with (
            tc.tile_pool(name="sb", bufs=1) as sb,
            tc.tile_pool(name="lp", bufs=4) as lpp,
            tc.tile_pool(name="mm", bufs=4) as mmp,
            tc.tile_pool(name="psum", bufs=6, space="PSUM") as psum,
            tc.tile_pool(name="psum1", bufs=2, space="PSUM") as psum1,
        ):
            # -------- head: input DMAs on the earliest-starting queues ------
            xt = sb.tile([47, 84], dt.float32)
            nc.vector.memset(xt[:], 0.0)
            nc.sync.dma_start(xt[:, 2:82], xs_d[:])       # SP queue (first)
            b5t = sb.tile([47, ROWS], dt.float32)
            nc.sync.dma_start(b5t[:], b5_d[:])            # SP queue
            xm = sb.tile([ROWS, 80], dt.float32)
            nc.gpsimd.dma_start(xm[:], xm_d[:])           # Pool queue
            cons = sb.tile([128, 4], dt.float32)
            nc.gpsimd.dma_start(cons[:], cons_d[:])       # Pool queue
            km = sb.tile([128, 640], dt.float16)
            nc.scalar.dma_start(km[:], kmI_d[:])          # ACT queue
            binsA = cons[:, 0:1]
            binsB = cons[:, 1:2]
            eps_t = cons[:, 2:3]

            onesrow = sb.tile([1, 128], dt.float16)
            nc.vector.memset(onesrow[:], 1.0)
            ones = sb.tile([128, 1], dt.float16)
            nc.vector.memset(ones[:], 1.0)
            warm = sb.tile([128, 1], dt.float32)
            nc.vector.memset(warm[:], 1.0)
            wout = sb.tile([128, 1], dt.float32)
            # Ln act-table warm-up while ACT is idle (table load ~1.3us)
            nc.scalar.activation(wout[:], warm[:], Act.Ln, bias=1.0)

            # -------- stage A: unsharp preprocessing, col-half ping-pong ----
            sv_ps = psum1.tile([ROWS, 84], dt.float32, tag="mps")
            nc.tensor.matmul(sv_ps[:], b5t[:], xt[:], start=True, stop=True)
            sv = sb.tile([ROWS, 84], dt.float32)
            nc.scalar.copy(sv[:], sv_ps[:])

            t1 = sb.tile([ROWS, 83], dt.float32)
            for c0, c1 in ((0, 44), (44, 83)):
                nc.vector.tensor_add(
                    t1[:, c0:c1], sv_ps[:, c0:c1], sv[:, c0 + 1 : c1 + 1]
                )
            t2 = sb.tile([ROWS, 81], dt.float32)
            for c0, c1 in ((0, 42), (42, 81)):
                nc.vector.tensor_add(
                    t2[:, c0:c1], t1[:, c0:c1], t1[:, c0 + 2 : c1 + 2]
                )
            s25 = sb.tile([ROWS, 80], dt.float32)
            for c0, c1 in ((0, 40), (40, 80)):
                nc.vector.tensor_add(
                    s25[:, c0:c1], t2[:, c0:c1], sv_ps[:, c0 + 4 : c1 + 4]
                )

            def pp(fn):
                for c0, c1 in ((0, 40), (40, 80)):
                    fn(slice(c0, c1))

            tt = sb.tile([ROWS, 80], dt.float32)
            pp(lambda s_: nc.vector.tensor_scalar(
                tt[:, s_], s25[:, s_], INV25, MAGIC, Alu.mult, Alu.add))
            smooth = sb.tile([ROWS, 80], dt.float32)
            pp(lambda s_: nc.vector.tensor_scalar(
                smooth[:, s_], tt[:, s_], MAGIC, None, Alu.subtract))

            denom = sb.tile([ROWS, 80], dt.float32)
            pp(lambda s_: nc.vector.tensor_scalar(
                denom[:, s_], smooth[:, s_], 1e-8, None, Alu.add))
            rr = sb.tile([ROWS, 80], dt.float32)
            pp(lambda s_: nc.vector.reciprocal(rr[:, s_], denom[:, s_]))

            sp = sb.tile([ROWS, 80], dt.float32)
            pp(lambda s_: nc.vector.scalar_tensor_tensor(
                sp[:, s_], smooth[:, s_], -1.25, xm[:, s_], Alu.mult, Alu.add))
            pp(lambda s_: nc.vector.tensor_scalar(
                sp[:, s_], sp[:, s_], 0.0, 255.0, Alu.max, Alu.min))
            pp(lambda s_: nc.vector.tensor_scalar(
                tt[:, s_], sp[:, s_], MAGIC, None, Alu.add))
            sharp = sb.tile([ROWS, 80], dt.float32)
            pp(lambda s_: nc.vector.tensor_scalar(
                sharp[:, s_], tt[:, s_], MAGIC, None, Alu.subtract))

            vv = sb.tile([ROWS, 80], dt.float32)
            pp(lambda s_: nc.vector.scalar_tensor_tensor(
                vv[:, s_], sharp[:, s_], 255.0, rr[:, s_], Alu.mult, Alu.mult))
            pp(lambda s_: nc.vector.tensor_scalar(
                vv[:, s_], vv[:, s_], MAGIC, None, Alu.add))
            dv = sb.tile([ROWS, 80], dt.float16)
            pp(lambda s_: nc.vector.tensor_scalar(
                dv[:, s_], vv[:, s_], MAGIC, 255.0, Alu.subtract, Alu.min))

            dvrow = sb.tile([1, NPIX], dt.float16)
            nc.sync.dma_start(dvrow[:], dv[:])            # SP queue

            # -------- broadcast + one-hot + trees + stage C -----------------
            dv_bc = sb.tile([128, NPIX], dt.float16, tag="dv_bc")
            oh = sb.tile([128, 2 * NPIX], dt.float16, tag="oh")
            bc_chunks = []
            boff = 0
            while boff < NPIX:
                bw = min(BCCH, NPIX - boff)
                bc_chunks.append((boff, bw))
                boff += bw

            bc_ps0 = None
            for k, (off, bw) in enumerate(bc_chunks):
                bc_ps = psum.tile([128, bw], dt.float32, tag="g_ps", name="bc")
                nc.tensor.matmul(
                    bc_ps[:], onesrow[:], dvrow[:, off : off + bw],
                    start=True, stop=True,
                )
                if k == 0:
                    bc_ps0 = bc_ps   # chunk 0 is_eq reads PSUM directly
                else:
                    nc.scalar.copy(dv_bc[:, off : off + bw], bc_ps[:])

            def is_eq_group(g0, g1, src=None):
                src = src if src is not None else dv_bc[:, g0:g1]
                nc.vector.tensor_scalar(
                    oh[:, g0:g1], src, binsA, None, Alu.is_equal
                )
                nc.vector.tensor_scalar(
                    oh[:, NPIX + g0 : NPIX + g1], src, binsB, None, Alu.is_equal,
                )

            v1 = sb.tile([128, 2 * 41 * 80], dt.float16, tag="v1")
            v2 = sb.tile([128, 2 * 39 * 80], dt.float16, tag="v2")
            u2 = sb.tile([128, 2 * PR * 80], dt.float16, tag="u2")
            v7 = sb.tile([128, 2 * PR * 80], dt.float16, tag="v7")
            t1h = sb.tile([128, 2 * PR * 79], dt.float16, tag="t1h")
            t2h = sb.tile([128, 2 * PR * 77], dt.float16, tag="t2h")
            uh = sb.tile([128, 2 * PR * HP], dt.float16, tag="uh")
            h_f = sb.tile([128, 2 * NP_], dt.float16, tag="h_f")

            oh4 = oh[:].rearrange("p (h r c) -> p h r c", h=2, r=ROWS, c=80)
            v1v = v1[:].rearrange("p (h r c) -> p h r c", h=2, r=41, c=80)
            v2v = v2[:].rearrange("p (h r c) -> p h r c", h=2, r=39, c=80)
            u2v = u2[:].rearrange("p (h r c) -> p h r c", h=2, r=PR, c=80)
            v7v = v7[:].rearrange("p (h r c) -> p h r c", h=2, r=PR, c=80)
            t1v = t1h[:].rearrange("p (h r c) -> p h r c", h=2, r=PR, c=79)
            t2v = t2h[:].rearrange("p (h r c) -> p h r c", h=2, r=PR, c=77)
            uhv = uh[:].rearrange("p (h r c) -> p h r c", h=2, r=PR, c=HP)
            hfv = h_f[:].rearrange("p (h r c) -> p h r c", h=2, r=PR, c=HP)

            def v_level(dstv, s1v, sh1, s2v, sh2, lo, hi):
                # one op per bin-half: contiguous slices, consecutive ops
                # independent (ping-pong) so the DVE never stalls on acks
                for h in (slice(0, 1), slice(1, 2)):
                    nc.vector.tensor_add(
                        dstv[:, h, lo:hi, :],
                        s1v[:, h, lo + sh1 : hi + sh1, :],
                        s2v[:, h, lo + sh2 : hi + sh2, :],
                    )

            def v_block(a, b, first=False):
                a1, b1 = (a if first else a + 4), min(b + 4, 41)
                a2, b2 = (a if first else a + 2), min(b + 2, 39)
                v_level(v1v, oh4, 0, oh4, 1, a1, b1)
                v_level(v2v, v1v, 0, v1v, 2, a2, b2)
                v_level(u2v, v2v, 0, v1v, 4, a, b)
                v_level(v7v, u2v, 0, oh4, 6, a, b)

            def h_group(a, b):
                for h in (slice(0, 1), slice(1, 2)):
                    nc.vector.tensor_add(
                        t1v[:, h, a:b, :], v7v[:, h, a:b, 0:79],
                        v7v[:, h, a:b, 1:80],
                    )
                for h in (slice(0, 1), slice(1, 2)):
                    nc.vector.tensor_add(
                        t2v[:, h, a:b, :], t1v[:, h, a:b, 0:77],
                        t1v[:, h, a:b, 2:79],
                    )
                for h in (slice(0, 1), slice(1, 2)):
                    nc.vector.tensor_add(
                        uhv[:, h, a:b, :], t2v[:, h, a:b, 0:HP],
                        t1v[:, h, a:b, 4 : 4 + HP],
                    )
                for h in (slice(0, 1), slice(1, 2)):
                    nc.vector.tensor_add(
                        hfv[:, h, a:b, :], uhv[:, h, a:b, :],
                        v7v[:, h, a:b, 6:80],
                    )

            def stage_c_pe_act(off, cw):
                h0c = h_f[:, off : off + cw]
                h1c = h_f[:, NP_ + off : NP_ + off + cw]
                g0 = psum.tile([128, cw], dt.float32, tag="g_ps", name="g0")
                nc.tensor.matmul(g0[:], km[:, 0:128], h0c, start=True, stop=False)
                nc.tensor.matmul(g0[:], km[:, 128:256], h1c, start=False, stop=True)
                g1 = psum.tile([128, cw], dt.float32, tag="g_ps", name="g1")
                nc.tensor.matmul(g1[:], km[:, 256:384], h0c, start=True, stop=False)
                nc.tensor.matmul(g1[:], km[:, 384:512], h1c, start=False, stop=True)
                lp0 = lpp.tile([128, cw], dt.float16, tag="lp0", name="lp0")
                nc.scalar.activation(lp0[:], g0[:], Act.Ln, bias=eps_t, scale=LN_SCALE)
                lp1 = lpp.tile([128, cw], dt.float16, tag="lp1", name="lp1")
                nc.scalar.activation(lp1[:], g1[:], Act.Ln, bias=eps_t, scale=LN_SCALE)
                return lp0, lp1

            def stage_c_muls(off, cw, lp0, lp1):
                h0c = h_f[:, off : off + cw]
                h1c = h_f[:, NP_ + off : NP_ + off + cw]
                m0 = mmp.tile([128, cw], dt.float16, tag="m0", name="m0")
                nc.vector.tensor_mul(m0[:], h0c, lp0[:])
                m1 = mmp.tile([128, cw], dt.float16, tag="m1", name="m1")
                nc.vector.tensor_mul(m1[:], h1c, lp1[:])
                return m0, m1

            def stage_c_out(off, cw, m0, m1):
                e_ps = psum1.tile([1, cw], dt.float32, tag="mps")
                nc.tensor.matmul(e_ps[:], ones[:], m0[:], start=True, stop=False)
                nc.tensor.matmul(e_ps[:], ones[:], m1[:], start=False, stop=True)
                ent_c = lpp.tile([1, cw], dt.float32, tag="entc", name="entc")
                nc.scalar.activation(
                    ent_c[:], e_ps[:], Act.Copy, bias=0.0, scale=NEG_INV_L
                )
                nc.sync.dma_start(ent_d[off : off + cw], ent_c[:])

            # ---- interleaved emission ----
            chunks = [(HP * a, HP * (b - a)) for a, b in C_ROWS]

            is_eq_group(0, bc_chunks[0][1], src=bc_ps0[:])
            is_eq_group(bc_chunks[0][1], OH_GROUPS[1])
            is_eq_group(OH_GROUPS[1], OH_GROUPS[2])
            v_block(*V_BLOCKS[0], first=True)
            is_eq_group(OH_GROUPS[2], OH_GROUPS[3])
            # keep the PE continuously busy while the v-tree runs so it holds
            # max pstate when the tap lane starts (idle resets the clock ramp)
            warm_ps = psum1.tile([1, 512], dt.float32, tag="mps", name="warm")
            for _ in range(20):
                nc.tensor.matmul(
                    warm_ps[:], ones[:], oh[0:128, 0:512], start=True, stop=True
                )
            for r0 in range(0, 12, TAP_BATCH):        # PE h-taps rows [0,12)
                h_taps(r0, TAP_BATCH)
            lps_a = [stage_c_pe_act(off, cw) for off, cw in chunks[0:2]]
            v_block(*V_BLOCKS[1])
            for r0 in range(12, PE_ROWS, TAP_BATCH):  # PE h-taps rows [12,24)
                h_taps(r0, TAP_BATCH)
            lps_b = [stage_c_pe_act(off, cw) for off, cw in chunks[2:4]]
            v_block(*V_BLOCKS[2])
            ms_a = [stage_c_muls(off, cw, lp)
                    for (off, cw), lp in zip(chunks[0:2], lps_a)]
            h_group(PE_ROWS, 34)                      # DVE h rows [24,34)
            v_block(*V_BLOCKS[3])
            ms_b = [stage_c_muls(off, cw, lp)
                    for (off, cw), lp in zip(chunks[2:4], lps_b)]
            lps_c = [stage_c_pe_act(off, cw) for off, cw in chunks[4:6]]
            h_group(34, PR)                           # DVE h rows [34,37)
            lps_d = [stage_c_pe_act(off, cw) for off, cw in chunks[6:]]
            for (off, cw), m in zip(chunks[0:2], ms_a):
                stage_c_out(off, cw, m)
            for (off, cw), m in zip(chunks[2:4], ms_b):
                stage_c_out(off, cw, m)
            ms_c = [stage_c_muls(off, cw, lp)
                    for (off, cw), lp in zip(chunks[4:6], lps_c)]
            ms_d = [stage_c_muls(off, cw, lp)
                    for (off, cw), lp in zip(chunks[6:], lps_d)]
            for (off, cw), m in zip(chunks[4:6], ms_c):
                stage_c_out(off, cw, m)
            for (off, cw), m in zip(chunks[6:], ms_d):
                stage_c_out(off, cw, m)

    nc.compile()
    return nc


def _get_compiled():
    global _COMPILED
    if _COMPILED is None:
        _COMPILED = (_build_nc(), _host_constants())
    return _COMPILED


def _in_map_for_core(xi, core, consts):
    b, half = core // 2, core % 2
    r0 = half * PR
    strip = np.zeros((47, 80), f32)
    lo, hi = r0 - 2, r0 + 45
    slo, shi = max(lo, 0), min(hi, 80)
    strip[slo - lo : shi - lo] = xi[b, slo:shi]
    m = dict(consts)
    m["xs"] = strip
    m["xm"] = np.ascontiguousarray(f32(2.5) * strip[2 : 2 + ROWS])
    return m


def _run(x, trace=False, **kw):
    """x: (2,2,1,80,80) float32. Returns BassKernelResults."""
    nc, consts = _get_compiled()
    xi = np.ascontiguousarray(np.asarray(x, f32).reshape(4, 80, 80))
    in_maps = [_in_map_for_core(xi, core, consts) for core in range(8)]
    res = run_bass_kernel_spmd(nc, in_maps, list(range(8)), trace=trace, **kw)
    return res


def kernel(x):
    res = _run(x)
    out = np.zeros((4, 80, 80), f32)
    pad = R // 2
    for core in range(8):
        b, half = core // 2, core % 2
        r0 = half * PR
        ent = np.asarray(res.results[core]["ent"], f32).reshape(PR, HP)
        out[b, pad + r0 : pad + r0 + PR, pad : pad + HP] = ent
    return out.reshape(2, 2, 80, 80)
